# revision 1
# baseline (speedup 1.0000x reference)
"""Multi-head causal attention (B=1, T=4096, D=768, H=12) on 8 trn2 cores.

Sharding: 16 uniform head-slots (2 per core), 12 real heads + 4 dummy
(zero-weight) slots.  Every core runs the IDENTICAL program (SPMD); cores
differ only in the weight data they receive.  Each core computes, for its
two head-slots, the full causal attention over all 4096 tokens plus that
slot-pair's partial output projection (out.T = wo_slice.T @ headout).  The
host sums the 8 partial [768, 4096] outputs, transposes, and adds the
output bias.

On-device layout (per core):
  xT   [768, 4096]  bf16   x transposed (host supplies)
  QT/KT[128, 4096]  bf16   partitions 0:64 head A dims, 64:128 head B dims
  VT   [128, 4096]  bf16   same, then PE-transposed to V [tok, dims]
  scoresT chunks [128 keys, 256 queries] in PSUM, exp'd on ACT -> PT bf16
  AV + denominators accumulated in PSUM via (V | ones) packed matmuls
"""

import math
import os
import numpy as np
import ml_dtypes
from contextlib import ExitStack

import concourse.bass as bass
import concourse.bacc as bacc
import concourse.mybir as mybir
import concourse.tile as tile
from concourse.bass_utils import run_bass_kernel_spmd

BF16 = mybir.dt.bfloat16
F32 = mybir.dt.float32
AF = mybir.ActivationFunctionType

T = 4096
D_MODEL = 768
HEAD_DIM = 64
N_HEADS = 12
N_CORES = 8
QT = 512                  # query tile width (one full PSUM bank per chunk)
KC = 128                  # key chunk (psum partition dim)
GRP = 3                   # score chunk-jobs per exp group -> ACT free dim 1536
NQT = T // QT             # 8 query tiles
CCH = D_MODEL // 128      # 6 contraction chunks
TOKT = 512                # token tile for projections
NTOKT = T // TOKT

_PROGRAM_CACHE = {}


def build_program(n_qtiles=NQT, skip_attn=False, attn_stage=7):
    nc = bacc.Bacc(None)

    xT_d = nc.declare_dram_parameter("xT", [D_MODEL, T], BF16, isOutput=False)
    w_d = nc.declare_dram_parameter("wqkv", [3, D_MODEL, 128], BF16, isOutput=False)
    b_d = nc.declare_dram_parameter("bqkv", [128, 3], F32, isOutput=False)
    wo_d = nc.declare_dram_parameter("wo2", [128, D_MODEL], BF16, isOutput=False)
    mk_d = nc.declare_dram_parameter("masks", [4, 128, QT], BF16, isOutput=False)
    id_d = nc.declare_dram_parameter("ident", [128, 128], BF16, isOutput=False)
    outT_d = nc.declare_dram_parameter("outT", [D_MODEL, T], F32, isOutput=True)

    with tile.TileContext(nc) as tc, ExitStack() as ctx:
        consts = ctx.enter_context(tc.tile_pool(name="consts", bufs=1))
        big = ctx.enter_context(tc.tile_pool(name="big", bufs=1))
        ptp = ctx.enter_context(tc.tile_pool(name="ptp", bufs=3))
        rp = ctx.enter_context(tc.tile_pool(name="rp", bufs=2))
        osb = ctx.enter_context(tc.tile_pool(name="osb", bufs=3))
        # PSUM: score/proj/outproj pool 3 banks x2, av 2 banks x1 = 8 banks
        sp = ctx.enter_context(tc.tile_pool(name="sp", bufs=2, space="PSUM"))
        avp = ctx.enter_context(tc.tile_pool(name="avp", bufs=1, space="PSUM"))
        dramp = ctx.enter_context(tc.tile_pool(name="dramp", bufs=2, space="DRAM"))

        # ---- constants / inputs to SBUF ----
        xT_sb = []
        for j in range(CCH):
            t = big.tile([128, T], BF16, tag=f"xT{j}")
            nc.sync.dma_start(out=t[:], in_=xT_d[j * 128:(j + 1) * 128, :])
            xT_sb.append(t)
        w_sb = consts.tile([128, 3 * CCH * 128], BF16, tag="w")
        for s in range(3):
            for j in range(CCH):
                nc.sync.dma_start(
                    out=w_sb[:, (s * CCH + j) * 128:(s * CCH + j + 1) * 128],
                    in_=w_d[s, j * 128:(j + 1) * 128, :],
                )
        b_sb = consts.tile([128, 3], F32, tag="b")
        nc.sync.dma_start(out=b_sb[:], in_=b_d[:, :])
        wo_sb = consts.tile([128, D_MODEL], BF16, tag="wo")
        nc.sync.dma_start(out=wo_sb[:], in_=wo_d[:, :])
        mask_sb = consts.tile([128, 4 * QT], BF16, tag="mask")
        for p in range(4):
            nc.sync.dma_start(out=mask_sb[:, p * QT:(p + 1) * QT], in_=mk_d[p, :, :])
        id_sb = consts.tile([128, 128], BF16, tag="id")
        nc.sync.dma_start(out=id_sb[:], in_=id_d[:, :])

        # ---- projections: QT/KT/VT [128(A|B dims), T] ----
        qkv_sb = []
        for s in range(3):
            t = big.tile([128, T], BF16, tag=f"qkv{s}")
            qkv_sb.append(t)
        for s in range(3):
            for tt in range(NTOKT):
                # head-A accumulation group in bank 0, head-B in bank 1
                pp = sp.tile([128, 2 * TOKT], F32, tag="sc")
                for j in range(CCH):
                    base = (s * CCH + j) * 128
                    rhs = xT_sb[j][:, tt * TOKT:(tt + 1) * TOKT]
                    nc.tensor.matmul(
                        pp[0:64, 0:TOKT], w_sb[:, base:base + 64], rhs,
                        start=(j == 0), stop=(j == CCH - 1), tile_position=(0, 0),
                    )
                    nc.tensor.matmul(
                        pp[64:128, TOKT:2 * TOKT], w_sb[:, base + 64:base + 128], rhs,
                        start=(j == 0), stop=(j == CCH - 1), tile_position=(0, 64),
                    )
                nc.vector.tensor_scalar_add(
                    qkv_sb[s][0:64, tt * TOKT:(tt + 1) * TOKT],
                    pp[0:64, 0:TOKT], b_sb[0:64, s:s + 1],
                )
                nc.vector.tensor_scalar_add(
                    qkv_sb[s][64:128, tt * TOKT:(tt + 1) * TOKT],
                    pp[64:128, TOKT:2 * TOKT], b_sb[64:128, s:s + 1],
                )
        QT_sb, KT_sb, VT_sb = qkv_sb

        # ---- V2 per 128-token key tile, stride 208 cols:
        #   [0:64]=V_A  [64:65]=1  [97:98]=1  [129:193]=V_B  (rest 0)
        # lhsT A = cols 0:65  -> psum rows 0:64 AV_A, row 64 denom_A
        # lhsT B = cols 65:193 -> psum row 32 denom_B, rows 64:128 AV_B
        #                         (single accumulation group per bank)
        VST = 208
        V_sb = big.tile([128, (T // 128) * VST], BF16, tag="V")
        nc.vector.memset(V_sb[:], 0.0)
        v3 = V_sb[:].rearrange("p (t c) -> p t c", c=VST)
        nc.vector.memset(v3[:, :, 64:65], 1.0)
        nc.vector.memset(v3[:, :, 97:98], 1.0)
        for tt4 in range(T // 128):
            tp = sp.tile([128, 128], BF16, tag="sc")
            nc.tensor.transpose(tp[:], VT_sb[:, tt4 * 128:(tt4 + 1) * 128], id_sb[:])
            nc.vector.tensor_copy(V_sb[:, tt4 * VST:tt4 * VST + 64], tp[:, 0:64])
            nc.vector.tensor_copy(
                V_sb[:, tt4 * VST + 129:tt4 * VST + 193], tp[:, 64:128])

        # ---- attention + out-projection ----
        ho_all = big.tile([128, T], BF16, tag="ho")
        if skip_attn:
            nc.vector.memset(ho_all[:], 0.0)
        for qi in range(n_qtiles):
            qs = qi * QT
            if skip_attn:
                for dch in range(CCH):
                    op = sp.tile([128, QT], F32, tag="sc")
                    nc.tensor.matmul(
                        op[:], wo_sb[:, dch * 128:(dch + 1) * 128],
                        ho_all[:, qs:qs + QT], start=True, stop=True,
                    )
                    ot = osb.tile([128, QT], F32, tag="ot")
                    nc.vector.tensor_copy(ot[:], op[:])
                    nc.sync.dma_start(
                        out=outT_d[dch * 128:(dch + 1) * 128, qs:qs + QT], in_=ot[:])
                continue
            # av bank 0: head-A group (AV rows 0:64, denom row 64)
            # av bank 1: head-B group (denom row 32, AV rows 64:128)
            av = None
            if attn_stage >= 4:
                av = avp.tile([128, 2 * QT], F32, tag="av")
            nsteps = 4 * (qi + 1)
            # one chunk job = [128 keys x QT queries] scores for one head
            # = exactly one PSUM bank; jobs grouped GRP at a time for exp
            jobs = [(kc, h) for kc in range(nsteps) for h in (0, 1)]
            for g in range(0, len(jobs), GRP):
                grp = jobs[g:g + GRP]
                width = len(grp) * QT
                sc = sp.tile([128, GRP * QT], F32, tag="sc")
                for ji, (kc, h) in enumerate(grp):
                    nc.tensor.matmul(
                        sc[:, ji * QT:(ji + 1) * QT],
                        KT_sb[64 * h:64 * h + 64, kc * KC:(kc + 1) * KC],
                        QT_sb[64 * h:64 * h + 64, qs:qs + QT],
                        start=True, stop=True, tile_position=(64 * h, 0),
                    )
                if attn_stage < 2:
                    continue
                pt = ptp.tile([128, GRP * QT], BF16, tag="pt")
                nc.scalar.activation(
                    pt[:, :width], sc[:, :width], AF.Exp,
                    scale=1.0 / math.sqrt(HEAD_DIM),
                )
                for ji, (kc, h) in enumerate(grp):
                    ptj = pt[:, ji * QT:(ji + 1) * QT]
                    if attn_stage >= 3 and kc >= 4 * qi:  # diagonal straddle
                        pat = kc - 4 * qi
                        m = mask_sb[:, pat * QT:(pat + 1) * QT]
                        nc.vector.tensor_mul(ptj, ptj, m)
                    if attn_stage < 4:
                        continue
                    st = kc == 0
                    sp_ = kc == nsteps - 1
                    vbase = kc * 208
                    if h == 0:
                        nc.tensor.matmul(
                            av[0:65, 0:QT], V_sb[:, vbase:vbase + 65], ptj,
                            start=st, stop=sp_, tile_position=(0, 0),
                        )
                    else:
                        nc.tensor.matmul(
                            av[0:128, QT:2 * QT], V_sb[:, vbase + 65:vbase + 193], ptj,
                            start=st, stop=sp_, tile_position=(0, 0),
                        )
            if attn_stage < 5:
                continue
            # normalize: recip of denominators, partition-broadcast via DRAM
            r = rp.tile([128, 2 * QT], F32, tag="r")
            nc.vector.reciprocal(r[64:65, 0:QT], av[64:65, 0:QT])
            nc.vector.reciprocal(r[32:33, QT:2 * QT], av[32:33, QT:2 * QT])
            # partition-broadcast via DRAM bounce (stride-0 partition reads
            # are only legal from DRAM)
            if attn_stage < 6:
                continue
            rd = dramp.tile([1, 2 * QT], F32, tag="rd")
            nc.sync.dma_start(out=rd[0:1, 0:QT], in_=r[64:65, 0:QT])
            nc.sync.dma_start(out=rd[0:1, QT:2 * QT], in_=r[32:33, QT:2 * QT])
            rbc = rp.tile([128, QT], F32, tag="rbc")
            rdA = rd[0:1, 0:QT]
            rdB = rd[0:1, QT:2 * QT]
            nc.gpsimd.dma_start(
                out=rbc[0:64, :],
                in_=bass.AP(tensor=rdA.tensor, offset=rdA.offset,
                            ap=[[0, 64]] + list(rdA.ap[1:])))
            nc.gpsimd.dma_start(
                out=rbc[64:128, :],
                in_=bass.AP(tensor=rdB.tensor, offset=rdB.offset,
                            ap=[[0, 64]] + list(rdB.ap[1:])))
            if attn_stage < 7:
                continue
            nc.vector.tensor_mul(
                ho_all[0:64, qs:qs + QT], av[0:64, 0:QT], rbc[0:64, :])
            nc.vector.tensor_mul(
                ho_all[64:128, qs:qs + QT], av[64:128, QT:2 * QT], rbc[64:128, :])
            # out projection for this query tile: outT[dout, q]
            for dch in range(CCH):
                op = sp.tile([128, QT], F32, tag="sc")
                nc.tensor.matmul(
                    op[:], wo_sb[:, dch * 128:(dch + 1) * 128],
                    ho_all[:, qs:qs + QT], start=True, stop=True,
                )
                ot = osb.tile([128, QT], F32, tag="ot")
                nc.vector.tensor_copy(ot[:], op[:])
                nc.sync.dma_start(
                    out=outT_d[dch * 128:(dch + 1) * 128, qs:qs + QT], in_=ot[:],
                )
    nc.finalize()
    return nc


def _host_inputs(x, wq, bq, wk, bk, wv, bv, wo):
    """Per-core input maps. Slot A of core c = head c; slot B = head 8+c
    (cores 0-3) or a dummy zero head (cores 4-7)."""
    bf16 = ml_dtypes.bfloat16
    xT = np.ascontiguousarray(x[0].T).astype(bf16)
    masks = np.zeros((4, 128, QT), np.float32)
    dk = np.arange(128)[:, None]
    dq = np.arange(QT)[None, :]
    for p in range(4):
        masks[p] = (dk + 128 * p <= dq)
    masks = masks.astype(bf16)
    ident = np.eye(128, dtype=np.float32).astype(bf16)

    in_maps = []
    for c in range(N_CORES):
        hA = c
        hB = 8 + c if c < 4 else None
        w = np.zeros((3, D_MODEL, 128), np.float32)
        b = np.zeros((128, 3), np.float32)
        wo2 = np.zeros((128, D_MODEL), np.float32)
        for s, (W, B) in enumerate(((wq, bq), (wk, bk), (wv, bv))):
            w[s, :, 0:64] = W[hA]
            b[0:64, s] = B[hA]
            if hB is not None:
                w[s, :, 64:128] = W[hB]
                b[64:128, s] = B[hB]
        wo2[0:64, :] = wo[hA * 64:(hA + 1) * 64, :]
        if hB is not None:
            wo2[64:128, :] = wo[hB * 64:(hB + 1) * 64, :]
        in_maps.append({
            "xT": xT,
            "wqkv": w.astype(bf16),
            "bqkv": b.astype(np.float32),
            "wo2": wo2.astype(bf16),
            "masks": masks,
            "ident": ident,
        })
    return in_maps


def kernel(_trace=False, _tmpdir=None, **inputs):
    x = np.asarray(inputs["x"], np.float32)
    args = (x,
            np.asarray(inputs["wq"], np.float32), np.asarray(inputs["bq"], np.float32),
            np.asarray(inputs["wk"], np.float32), np.asarray(inputs["bk"], np.float32),
            np.asarray(inputs["wv"], np.float32), np.asarray(inputs["bv"], np.float32),
            np.asarray(inputs["wo"], np.float32))
    bo = np.asarray(inputs["bo"], np.float32)

    if "nc" not in _PROGRAM_CACHE:
        _PROGRAM_CACHE["nc"] = build_program()
    nc = _PROGRAM_CACHE["nc"]

    in_maps = _host_inputs(*args)
    res = run_bass_kernel_spmd(
        nc, in_maps, list(range(N_CORES)), trace=_trace, tmpdir=_tmpdir,
    )
    acc = np.zeros((D_MODEL, T), np.float32)
    for c in range(N_CORES):
        acc += res.results[c]["outT"]
    out = acc.T + bo[None, :]
    if _trace:
        return out[None].astype(np.float32), res
    return out[None].astype(np.float32)



# revision 4
# speedup vs baseline: 1.1496x; 1.1496x over previous
"""Multi-head causal attention (B=1, T=4096, D=768, H=12) on 8 trn2 cores.

Sharding: 16 uniform head-slots (2 per core), 12 real heads + 4 dummy
(zero-weight) slots.  Every core runs the IDENTICAL program (SPMD); cores
differ only in the weight data they receive.  Each core computes, for its
two head-slots, the full causal attention over all 4096 tokens plus that
slot-pair's partial output projection (out.T = wo_slice.T @ headout).  The
host sums the 8 partial [768, 4096] outputs (fp16), transposes, and adds
the output bias.

On-device layout (per core):
  xT   [768, 4096]  bf16   x transposed (host supplies)
  QT/KT[128, 4096]  bf16   partitions 0:64 head A dims, 64:128 head B dims
  V2   [128, 32*208] bf16  per 128-key chunk: [V_A | 1 | .. 1(col97) | V_B]
                           built directly transposed by the projection
                           (lhsT = xT chunk), V bias folded in via a
                           rank-1 ones-row matmul
  scores chunks [128 keys, 512 queries] in PSUM, exp'd on ACT -> PT bf16
  AV + denominators accumulated in PSUM via the V2 packed matmuls
  normalization deferred: AV copied to SBUF f32, denominators batched
  through one [128, 16] reciprocal per 2 query tiles (DRAM bounce for the
  partition broadcast), then out-projection + fp16 store
"""

import math
import numpy as np
import ml_dtypes
from contextlib import ExitStack

import concourse.bass as bass
import concourse.bacc as bacc
import concourse.mybir as mybir
import concourse.tile as tile
from concourse.bass_utils import run_bass_kernel_spmd

BF16 = mybir.dt.bfloat16
F16 = mybir.dt.float16
F32 = mybir.dt.float32
AF = mybir.ActivationFunctionType

T = 4096
D_MODEL = 768
HEAD_DIM = 64
N_HEADS = 12
N_CORES = 8
QT = 512                  # query tile width (one full PSUM bank per chunk)
KC = 128                  # key chunk (psum partition dim)
GRP = 3                   # score chunk-jobs per exp group -> ACT free dim 1536
NQT = T // QT             # 8 query tiles
CCH = D_MODEL // 128      # 6 contraction chunks
TOKT = 512                # token tile for Q/K projections
NTOKT = T // TOKT
VST = 208                 # V2 column stride per key chunk

_PROGRAM_CACHE = {}


def build_program():
    nc = bacc.Bacc(None)

    xT_d = nc.declare_dram_parameter("xT", [D_MODEL, T], BF16, isOutput=False)
    w_d = nc.declare_dram_parameter("wqkv", [3, D_MODEL, 128], BF16, isOutput=False)
    b_d = nc.declare_dram_parameter("bqkv", [128, 3], F32, isOutput=False)
    bvr_d = nc.declare_dram_parameter("bvrow", [1, 128], BF16, isOutput=False)
    wo_d = nc.declare_dram_parameter("wo2", [128, D_MODEL], BF16, isOutput=False)
    mk_d = nc.declare_dram_parameter("masks", [4, 128, QT], BF16, isOutput=False)
    outT_d = nc.declare_dram_parameter("outT", [D_MODEL, T], F16, isOutput=True)

    with tile.TileContext(nc) as tc, ExitStack() as ctx:
        consts = ctx.enter_context(tc.tile_pool(name="consts", bufs=1))
        big = ctx.enter_context(tc.tile_pool(name="big", bufs=1))
        ptp = ctx.enter_context(tc.tile_pool(name="ptp", bufs=3))
        rp = ctx.enter_context(tc.tile_pool(name="rp", bufs=2))
        osb = ctx.enter_context(tc.tile_pool(name="osb", bufs=3))
        # PSUM: sp holds score (3 banks) / proj (1) / outproj (1) slots x2
        # bufs = 6 banks; av 2 banks x1 = 8 banks total
        sp = ctx.enter_context(tc.tile_pool(name="sp", bufs=2, space="PSUM"))
        avp = ctx.enter_context(tc.tile_pool(name="avp", bufs=1, space="PSUM"))
        dramp = ctx.enter_context(tc.tile_pool(name="dramp", bufs=2, space="DRAM"))

        # ---- constants / inputs to SBUF (small first, then xT chunks) ----
        w_sb = consts.tile([128, 3 * CCH * 128], BF16, tag="w")
        for s in range(3):
            for j in range(CCH):
                nc.sync.dma_start(
                    out=w_sb[:, (s * CCH + j) * 128:(s * CCH + j + 1) * 128],
                    in_=w_d[s, j * 128:(j + 1) * 128, :],
                )
        b_sb = consts.tile([128, 3], F32, tag="b")
        nc.sync.dma_start(out=b_sb[:], in_=b_d[:, :])
        bvr_sb = consts.tile([1, 128], BF16, tag="bvr")
        nc.sync.dma_start(out=bvr_sb[:], in_=bvr_d[:, :])
        wo_sb = consts.tile([128, D_MODEL], BF16, tag="wo")
        nc.sync.dma_start(out=wo_sb[:], in_=wo_d[:, :])
        mask_sb = consts.tile([128, 4 * QT], BF16, tag="mask")
        for p in range(4):
            nc.sync.dma_start(out=mask_sb[:, p * QT:(p + 1) * QT], in_=mk_d[p, :, :])
        ones_sb = consts.tile([1, 128], BF16, tag="ones1")
        nc.vector.memset(ones_sb[:], 1.0)
        xT_sb = []
        for j in range(CCH):
            t = big.tile([128, T], BF16, tag=f"xT{j}")
            nc.sync.dma_start(out=t[:], in_=xT_d[j * 128:(j + 1) * 128, :])
            xT_sb.append(t)

        # ---- Q/K projections: [128(A|B dims), T], single 128-wide matmuls --
        QT_sb = big.tile([128, T], BF16, tag="qsb")
        KT_sb = big.tile([128, T], BF16, tag="ksb")
        qk_dst = (QT_sb, KT_sb)
        for tt in range(NTOKT):
            for s in range(2):
                pp = sp.tile([128, TOKT], F32, tag="sc")
                rhs = xT_sb  # per-chunk below
                for j in range(CCH):
                    base = (s * CCH + j) * 128
                    nc.tensor.matmul(
                        pp[:], w_sb[:, base:base + 128],
                        xT_sb[j][:, tt * TOKT:(tt + 1) * TOKT],
                        start=(j == 0), stop=(j == CCH - 1),
                    )
                nc.vector.tensor_scalar_add(
                    qk_dst[s][:, tt * TOKT:(tt + 1) * TOKT],
                    pp[:], b_sb[:, s:s + 1],
                )

        # ---- V built directly transposed: V2 [128 tok, 32 chunks x 208] ----
        #   per chunk cols: [0:64]=V_A  [64:65]=1  [97:98]=1  [129:193]=V_B
        #   lhsT A = cols 0:65   -> psum rows 0:64 AV_A, row 64 denom_A
        #   lhsT B = cols 65:193 -> psum row 32 denom_B, rows 64:128 AV_B
        V2 = big.tile([128, (T // KC) * VST], BF16, tag="V2")
        nc.vector.memset(V2[:], 0.0)
        v3 = V2[:].rearrange("p (t c) -> p t c", c=VST)
        nc.vector.memset(v3[:, :, 64:65], 1.0)
        nc.vector.memset(v3[:, :, 97:98], 1.0)
        for tt4 in range(T // KC):
            vt = sp.tile([128, 128], F32, tag="sc")
            for j in range(CCH):
                base = (2 * CCH + j) * 128
                nc.tensor.matmul(
                    vt[:], xT_sb[j][:, tt4 * KC:(tt4 + 1) * KC],
                    w_sb[:, base:base + 128],
                    start=(j == 0), stop=False,
                )
            # V bias via rank-1 update: out[tok, dim] += 1 * bv[dim]
            nc.tensor.matmul(
                vt[:], ones_sb[:, 0:128], bvr_sb[:, :],
                start=False, stop=True,
            )
            nc.vector.tensor_copy(V2[:, tt4 * VST:tt4 * VST + 64], vt[:, 0:64])
            nc.vector.tensor_copy(
                V2[:, tt4 * VST + 129:tt4 * VST + 193], vt[:, 64:128])

        # ---- attention: per qtile scores -> exp -> mask -> AV -> stash ----
        ho_u = big.tile([128, T], F32, tag="ho_u")      # unnormalized AV
        hob = big.tile([128, T], BF16, tag="hob")       # normalized, bf16
        dn_st = big.tile([128, 2 * QT], F32, tag="dn")  # denom staging
        for qi in range(NQT):
            qs = qi * QT
            # av bank 0: head-A group (AV rows 0:64, denom row 64)
            # av bank 1: head-B group (denom row 32, AV rows 64:128)
            av = avp.tile([128, 2 * QT], F32, tag="av")
            nsteps = 4 * (qi + 1)
            jobs = [(kc, h) for kc in range(nsteps) for h in (0, 1)]
            for g in range(0, len(jobs), GRP):
                grp = jobs[g:g + GRP]
                width = len(grp) * QT
                sc = sp.tile([128, GRP * QT], F32, tag="sc")
                for ji, (kc, h) in enumerate(grp):
                    nc.tensor.matmul(
                        sc[:, ji * QT:(ji + 1) * QT],
                        KT_sb[64 * h:64 * h + 64, kc * KC:(kc + 1) * KC],
                        QT_sb[64 * h:64 * h + 64, qs:qs + QT],
                        start=True, stop=True, tile_position=(64 * h, 0),
                    )
                pt = ptp.tile([128, GRP * QT], BF16, tag="pt")
                nc.scalar.activation(
                    pt[:, :width], sc[:, :width], AF.Exp,
                    scale=1.0 / math.sqrt(HEAD_DIM),
                )
                for ji, (kc, h) in enumerate(grp):
                    ptj = pt[:, ji * QT:(ji + 1) * QT]
                    if kc >= 4 * qi:  # diagonal straddle
                        pat = kc - 4 * qi
                        m = mask_sb[:, pat * QT:(pat + 1) * QT]
                        nc.vector.tensor_mul(ptj, ptj, m)
                    st = kc == 0
                    sp_ = kc == nsteps - 1
                    vbase = kc * VST
                    if h == 0:
                        nc.tensor.matmul(
                            av[0:65, 0:QT], V2[:, vbase:vbase + 65], ptj,
                            start=st, stop=sp_, tile_position=(0, 0),
                        )
                    else:
                        nc.tensor.matmul(
                            av[0:128, QT:2 * QT], V2[:, vbase + 65:vbase + 193],
                            ptj, start=st, stop=sp_, tile_position=(0, 0),
                        )
            # stash unnormalized AV + denominators; free av quickly
            nc.vector.tensor_copy(ho_u[0:64, qs:qs + QT], av[0:64, 0:QT])
            nc.vector.tensor_copy(
                ho_u[64:128, qs:qs + QT], av[64:128, QT:2 * QT])
            dcol = (qi % 2) * QT
            nc.vector.tensor_copy(
                dn_st[64:65, dcol:dcol + QT], av[64:65, 0:QT])
            nc.vector.tensor_copy(
                dn_st[32:33, dcol:dcol + QT], av[32:33, QT:2 * QT])

            # ---- deferred normalize + out-projection per 2 qtiles ----
            if qi % 2 == 1:
                # batch the 4 denominator rows into [128, 16] for one recip
                rd = dramp.tile([2, 2 * QT], F32, tag="rd")
                nc.sync.dma_start(out=rd[0:1, :], in_=dn_st[64:65, 0:2 * QT])
                nc.sync.dma_start(out=rd[1:2, :], in_=dn_st[32:33, 0:2 * QT])
                dn_sb = rp.tile([128, 16], F32, tag="dnsb")
                nc.sync.dma_start(
                    out=dn_sb[:, 0:16],
                    in_=bass.AP(tensor=rd.tensor, offset=rd.offset,
                                ap=[[16, 128], [1, 16]]))
                dn_r = rp.tile([128, 16], F32, tag="dnr")
                nc.vector.reciprocal(dn_r[:, 0:16], dn_sb[:, 0:16])
                rr = dramp.tile([1, 4 * QT], F32, tag="rr")
                nc.sync.dma_start(
                    out=bass.AP(tensor=rr.tensor, offset=rr.offset,
                                ap=[[16, 128], [1, 16]]),
                    in_=dn_r[:, 0:16])
                # rr linear layout: [rA(q even) | rA(q odd) | rB(even) | rB(odd)]
                for sub in range(2):
                    q2 = qi - 1 + sub
                    q2s = q2 * QT
                    rbc = rp.tile([128, QT], F32, tag="rbc")
                    rA = rr[0:1, sub * QT:(sub + 1) * QT]
                    rB = rr[0:1, (2 + sub) * QT:(3 + sub) * QT]
                    nc.gpsimd.dma_start(
                        out=rbc[0:64, :],
                        in_=bass.AP(tensor=rA.tensor, offset=rA.offset,
                                    ap=[[0, 64]] + list(rA.ap[1:])))
                    nc.gpsimd.dma_start(
                        out=rbc[64:128, :],
                        in_=bass.AP(tensor=rB.tensor, offset=rB.offset,
                                    ap=[[0, 64]] + list(rB.ap[1:])))
                    nc.vector.tensor_mul(
                        hob[:, q2s:q2s + QT], ho_u[:, q2s:q2s + QT], rbc[:])
                    for dch in range(CCH):
                        op = sp.tile([128, QT], F32, tag="sc")
                        nc.tensor.matmul(
                            op[:], wo_sb[:, dch * 128:(dch + 1) * 128],
                            hob[:, q2s:q2s + QT], start=True, stop=True,
                        )
                        ot = osb.tile([128, QT], F16, tag="ot")
                        nc.vector.tensor_copy(ot[:], op[:])
                        nc.sync.dma_start(
                            out=outT_d[dch * 128:(dch + 1) * 128,
                                       q2s:q2s + QT],
                            in_=ot[:],
                        )
    nc.finalize()
    return nc


def _host_inputs(x, wq, bq, wk, bk, wv, bv, wo):
    """Per-core input maps. Slot A of core c = head c; slot B = head 8+c
    (cores 0-3) or a dummy zero head (cores 4-7)."""
    bf16 = ml_dtypes.bfloat16
    xT = np.ascontiguousarray(x[0].T).astype(bf16)
    masks = np.zeros((4, 128, QT), np.float32)
    dk = np.arange(128)[:, None]
    dq = np.arange(QT)[None, :]
    for p in range(4):
        masks[p] = (dk + 128 * p <= dq)
    masks = masks.astype(bf16)

    in_maps = []
    for c in range(N_CORES):
        hA = c
        hB = 8 + c if c < 4 else None
        w = np.zeros((3, D_MODEL, 128), np.float32)
        b = np.zeros((128, 3), np.float32)
        bvrow = np.zeros((1, 128), np.float32)
        wo2 = np.zeros((128, D_MODEL), np.float32)
        for s, (W, B) in enumerate(((wq, bq), (wk, bk), (wv, bv))):
            w[s, :, 0:64] = W[hA]
            if s < 2:
                b[0:64, s] = B[hA]
            if hB is not None:
                w[s, :, 64:128] = W[hB]
                if s < 2:
                    b[64:128, s] = B[hB]
        bvrow[0, 0:64] = bv[hA]
        if hB is not None:
            bvrow[0, 64:128] = bv[hB]
        wo2[0:64, :] = wo[hA * 64:(hA + 1) * 64, :]
        if hB is not None:
            wo2[64:128, :] = wo[hB * 64:(hB + 1) * 64, :]
        in_maps.append({
            "xT": xT,
            "wqkv": w.astype(bf16),
            "bqkv": b.astype(np.float32),
            "bvrow": bvrow.astype(bf16),
            "wo2": wo2.astype(bf16),
            "masks": masks,
        })
    return in_maps


def kernel(_trace=False, _tmpdir=None, **inputs):
    x = np.asarray(inputs["x"], np.float32)
    args = (x,
            np.asarray(inputs["wq"], np.float32), np.asarray(inputs["bq"], np.float32),
            np.asarray(inputs["wk"], np.float32), np.asarray(inputs["bk"], np.float32),
            np.asarray(inputs["wv"], np.float32), np.asarray(inputs["bv"], np.float32),
            np.asarray(inputs["wo"], np.float32))
    bo = np.asarray(inputs["bo"], np.float32)

    if "nc" not in _PROGRAM_CACHE:
        _PROGRAM_CACHE["nc"] = build_program()
    nc = _PROGRAM_CACHE["nc"]

    in_maps = _host_inputs(*args)
    res = run_bass_kernel_spmd(
        nc, in_maps, list(range(N_CORES)), trace=_trace, tmpdir=_tmpdir,
    )
    acc = np.zeros((D_MODEL, T), np.float32)
    for c in range(N_CORES):
        acc += res.results[c]["outT"].astype(np.float32)
    out = acc.T + bo[None, :]
    if _trace:
        return out[None].astype(np.float32), res
    return out[None].astype(np.float32)


# revision 5
# speedup vs baseline: 1.3403x; 1.1659x over previous
"""Multi-head causal attention (B=1, T=4096, D=768, H=12) on 8 trn2 cores.

Sharding: 16 uniform head-slots (2 per core), 12 real heads + 4 dummy
(zero-weight) slots.  Every core runs the IDENTICAL program (SPMD); cores
differ only in the weight data they receive.  Each core computes, for its
two head-slots, the full causal attention over all 4096 tokens plus that
slot-pair's partial output projection (out.T = wo_slice.T @ headout).  The
host sums the 8 partial [768, 4096] fp16 outputs, transposes, adds bias.

Pipeline structure (v3): x arrives in 8 token blocks (block-major DRAM
layout so each block is one 128-line DMA).  Per block tt: project Q/K for
those 512 tokens, build V^T directly via transposed matmuls (lhsT = x
chunk), then run attention for query tile tt (its keys 0..tt are all
projected).  Softmax normalization is deferred and staged across later
block boundaries (denominator DMA bounce -> batched [128,16] reciprocal ->
partition-broadcast -> scale + out-projection) so no engine queue ever
waits on the DMA chain.
"""

import math
import numpy as np
import ml_dtypes
from contextlib import ExitStack

import concourse.bass as bass
import concourse.bacc as bacc
import concourse.mybir as mybir
import concourse.tile as tile
from concourse.bass_utils import run_bass_kernel_spmd

BF16 = mybir.dt.bfloat16
F16 = mybir.dt.float16
F32 = mybir.dt.float32
AF = mybir.ActivationFunctionType

T = 4096
D_MODEL = 768
HEAD_DIM = 64
N_HEADS = 12
N_CORES = 8
QT = 512                  # query tile width (one full PSUM bank per chunk)
KC = 128                  # key chunk (psum partition dim)
GRP = 3                   # score chunk-jobs per exp group -> ACT free dim 1536
NQT = T // QT             # 8 query tiles / token blocks
CCH = D_MODEL // 128      # 6 contraction chunks
BLK = QT * CCH            # 3072 cols per block in xTb layout
VST = 208                 # V2 column stride per key chunk

_PROGRAM_CACHE = {}


def build_program():
    nc = bacc.Bacc(None)

    # xTb block-major: [128, 8 blocks x (6 chunks x 512 tokens)]
    # xTb[p, tt*3072 + j*512 + i] = x[tt*512 + i, j*128 + p]
    xT_d = nc.declare_dram_parameter("xTb", [128, NQT * BLK], BF16, isOutput=False)
    w_d = nc.declare_dram_parameter("wqkv", [3, D_MODEL, 128], BF16, isOutput=False)
    b_d = nc.declare_dram_parameter("bqkv", [128, 3], F32, isOutput=False)
    bvr_d = nc.declare_dram_parameter("bvrow", [1, 128], BF16, isOutput=False)
    wo_d = nc.declare_dram_parameter("wo2", [128, D_MODEL], BF16, isOutput=False)
    mk_d = nc.declare_dram_parameter("masks", [4, 128, QT], BF16, isOutput=False)
    outT_d = nc.declare_dram_parameter("outT", [D_MODEL, T], F16, isOutput=True)

    with tile.TileContext(nc) as tc, ExitStack() as ctx:
        consts = ctx.enter_context(tc.tile_pool(name="consts", bufs=1))
        big = ctx.enter_context(tc.tile_pool(name="big", bufs=1))
        ptp = ctx.enter_context(tc.tile_pool(name="ptp", bufs=3))
        rp = ctx.enter_context(tc.tile_pool(name="rp", bufs=2))
        osb = ctx.enter_context(tc.tile_pool(name="osb", bufs=3))
        sp = ctx.enter_context(tc.tile_pool(name="sp", bufs=2, space="PSUM"))
        avp = ctx.enter_context(tc.tile_pool(name="avp", bufs=1, space="PSUM"))
        dramp = ctx.enter_context(tc.tile_pool(name="dramp", bufs=2, space="DRAM"))

        # ---- small constants first ----
        w_sb = consts.tile([128, 3 * CCH * 128], BF16, tag="w")
        for s in range(3):
            for j in range(CCH):
                nc.sync.dma_start(
                    out=w_sb[:, (s * CCH + j) * 128:(s * CCH + j + 1) * 128],
                    in_=w_d[s, j * 128:(j + 1) * 128, :],
                )
        b_sb = consts.tile([128, 3], F32, tag="b")
        nc.sync.dma_start(out=b_sb[:], in_=b_d[:, :])
        bvr_sb = consts.tile([1, 128], BF16, tag="bvr")
        nc.sync.dma_start(out=bvr_sb[:], in_=bvr_d[:, :])
        wo_sb = consts.tile([128, D_MODEL], BF16, tag="wo")
        nc.sync.dma_start(out=wo_sb[:], in_=wo_d[:, :])
        mask_sb = consts.tile([128, 4 * QT], BF16, tag="mask")
        for p in range(4):
            nc.sync.dma_start(out=mask_sb[:, p * QT:(p + 1) * QT], in_=mk_d[p, :, :])
        ones_sb = consts.tile([1, 128], BF16, tag="ones1")
        nc.vector.memset(ones_sb[:], 1.0)

        xT_sb = big.tile([128, NQT * BLK], BF16, tag="xTb")
        QT_sb = big.tile([128, T], BF16, tag="qsb")
        KT_sb = big.tile([128, T], BF16, tag="ksb")
        V2 = big.tile([128, (T // KC) * VST], BF16, tag="V2")
        nc.vector.memset(V2[:], 0.0)
        v3 = V2[:].rearrange("p (t c) -> p t c", c=VST)
        nc.vector.memset(v3[:, :, 64:65], 1.0)
        nc.vector.memset(v3[:, :, 97:98], 1.0)
        ho_u = big.tile([128, T], F32, tag="ho_u")      # unnormalized AV
        hob = big.tile([128, T], BF16, tag="hob")       # normalized, bf16
        dn_st = big.tile([128, 2 * QT], F32, tag="dn")  # denom staging

        rounds = {}  # round k -> dict of state tiles

        def xchunk(tt, j, lo, hi):
            base = tt * BLK + j * QT
            return xT_sb[:, base + lo:base + hi]

        def stage_a(k):
            rd = dramp.tile([2, 2 * QT], F32, tag="rd")
            nc.sync.dma_start(out=rd[0:1, :], in_=dn_st[64:65, 0:2 * QT])
            nc.sync.dma_start(out=rd[1:2, :], in_=dn_st[32:33, 0:2 * QT])
            dn_sb = rp.tile([128, 16], F32, tag="dnsb")
            nc.sync.dma_start(
                out=dn_sb[:, 0:16],
                in_=bass.AP(tensor=rd.tensor, offset=rd.offset,
                            ap=[[16, 128], [1, 16]]))
            rounds[k] = {"dn_sb": dn_sb}

        def stage_b(k):
            st = rounds[k]
            dn_r = rp.tile([128, 16], F32, tag="dnr")
            nc.vector.reciprocal(dn_r[:, 0:16], st["dn_sb"][:, 0:16])
            rr = dramp.tile([1, 4 * QT], F32, tag="rr")
            nc.sync.dma_start(
                out=bass.AP(tensor=rr.tensor, offset=rr.offset,
                            ap=[[16, 128], [1, 16]]),
                in_=dn_r[:, 0:16])
            # rr linear: [rA(q even) | rA(q odd) | rB(even) | rB(odd)]
            rbcs = []
            for sub in range(2):
                rbc = rp.tile([128, QT], F32, tag="rbc")
                rA = rr[0:1, sub * QT:(sub + 1) * QT]
                rB = rr[0:1, (2 + sub) * QT:(3 + sub) * QT]
                nc.gpsimd.dma_start(
                    out=rbc[0:64, :],
                    in_=bass.AP(tensor=rA.tensor, offset=rA.offset,
                                ap=[[0, 64]] + list(rA.ap[1:])))
                nc.gpsimd.dma_start(
                    out=rbc[64:128, :],
                    in_=bass.AP(tensor=rB.tensor, offset=rB.offset,
                                ap=[[0, 64]] + list(rB.ap[1:])))
                rbcs.append(rbc)
            st["rbcs"] = rbcs

        def stage_c(k):
            st = rounds.pop(k)
            for sub in range(2):
                q2s = (2 * k + sub) * QT
                nc.vector.tensor_mul(
                    hob[:, q2s:q2s + QT], ho_u[:, q2s:q2s + QT],
                    st["rbcs"][sub][:])
                for dch in range(CCH):
                    op = sp.tile([128, QT], F32, tag="sc")
                    nc.tensor.matmul(
                        op[:], wo_sb[:, dch * 128:(dch + 1) * 128],
                        hob[:, q2s:q2s + QT], start=True, stop=True,
                    )
                    ot = osb.tile([128, QT], F16, tag="ot")
                    nc.vector.tensor_copy(ot[:], op[:])
                    nc.sync.dma_start(
                        out=outT_d[dch * 128:(dch + 1) * 128, q2s:q2s + QT],
                        in_=ot[:],
                    )

        # ================= main token-block pipeline =================
        for tt in range(NQT):
            # block DMA: one 128-line transfer
            nc.sync.dma_start(
                out=xT_sb[:, tt * BLK:(tt + 1) * BLK],
                in_=xT_d[:, tt * BLK:(tt + 1) * BLK])

            # Q/K projections for this token block
            for s, dst in ((0, QT_sb), (1, KT_sb)):
                pp = sp.tile([128, QT], F32, tag="sc")
                for j in range(CCH):
                    nc.tensor.matmul(
                        pp[:], w_sb[:, (s * CCH + j) * 128:(s * CCH + j + 1) * 128],
                        xchunk(tt, j, 0, QT),
                        start=(j == 0), stop=(j == CCH - 1),
                    )
                nc.vector.tensor_scalar_add(
                    dst[:, tt * QT:(tt + 1) * QT], pp[:], b_sb[:, s:s + 1])

            # V^T for this block's 4 key chunks
            for q4 in range(4):
                tt4 = tt * 4 + q4
                vt = sp.tile([128, 128], F32, tag="sc")
                for j in range(CCH):
                    base = (2 * CCH + j) * 128
                    nc.tensor.matmul(
                        vt[:], xchunk(tt, j, q4 * KC, (q4 + 1) * KC),
                        w_sb[:, base:base + 128],
                        start=(j == 0), stop=False,
                    )
                nc.tensor.matmul(  # rank-1 bias: out[tok, dim] += bv[dim]
                    vt[:], ones_sb[:, 0:128], bvr_sb[:, :],
                    start=False, stop=True,
                )
                nc.vector.tensor_copy(V2[:, tt4 * VST:tt4 * VST + 64], vt[:, 0:64])
                nc.vector.tensor_copy(
                    V2[:, tt4 * VST + 129:tt4 * VST + 193], vt[:, 64:128])

            # ---- attention for query tile qi = tt ----
            qi = tt
            qs = qi * QT
            # av bank 0: head-A group (AV rows 0:64, denom row 64)
            # av bank 1: head-B group (denom row 32, AV rows 64:128)
            av = avp.tile([128, 2 * QT], F32, tag="av")
            nsteps = 4 * (qi + 1)
            jobs = [(kc, h) for kc in range(nsteps) for h in (0, 1)]
            for g in range(0, len(jobs), GRP):
                grp = jobs[g:g + GRP]
                width = len(grp) * QT
                sc = sp.tile([128, GRP * QT], F32, tag="sc")
                for ji, (kc, h) in enumerate(grp):
                    nc.tensor.matmul(
                        sc[:, ji * QT:(ji + 1) * QT],
                        KT_sb[64 * h:64 * h + 64, kc * KC:(kc + 1) * KC],
                        QT_sb[64 * h:64 * h + 64, qs:qs + QT],
                        start=True, stop=True, tile_position=(64 * h, 0),
                    )
                pt = ptp.tile([128, GRP * QT], BF16, tag="pt")
                nc.scalar.activation(
                    pt[:, :width], sc[:, :width], AF.Exp,
                    scale=1.0 / math.sqrt(HEAD_DIM),
                )
                for ji, (kc, h) in enumerate(grp):
                    ptj = pt[:, ji * QT:(ji + 1) * QT]
                    if kc >= 4 * qi:  # diagonal straddle
                        pat = kc - 4 * qi
                        m = mask_sb[:, pat * QT:(pat + 1) * QT]
                        nc.vector.tensor_mul(ptj, ptj, m)
                    st_ = kc == 0
                    sp_ = kc == nsteps - 1
                    vbase = kc * VST
                    if h == 0:
                        # lhsT padded to 128 cols (FWL); rows 65:128 junk
                        nc.tensor.matmul(
                            av[0:128, 0:QT], V2[:, vbase:vbase + 128], ptj,
                            start=st_, stop=sp_, tile_position=(0, 0),
                        )
                    else:
                        nc.tensor.matmul(
                            av[0:128, QT:2 * QT], V2[:, vbase + 65:vbase + 193],
                            ptj, start=st_, stop=sp_, tile_position=(0, 0),
                        )
            # stash unnormalized AV + denominators; free av quickly
            nc.vector.tensor_copy(ho_u[0:64, qs:qs + QT], av[0:64, 0:QT])
            nc.vector.tensor_copy(
                ho_u[64:128, qs:qs + QT], av[64:128, QT:2 * QT])
            dcol = (qi % 2) * QT
            nc.vector.tensor_copy(
                dn_st[64:65, dcol:dcol + QT], av[64:65, 0:QT])
            nc.vector.tensor_copy(
                dn_st[32:33, dcol:dcol + QT], av[32:33, QT:2 * QT])

            # staged deferred normalization + out-projection
            if qi % 2 == 1:
                stage_a(qi // 2)
            if qi >= 2 and qi % 2 == 0:
                stage_b(qi // 2 - 1)
            if qi >= 3 and qi % 2 == 1:
                stage_c(qi // 2 - 1)
        # drain remaining rounds
        stage_b(3)
        stage_c(3)
    nc.finalize()
    return nc


def _host_inputs(x, wq, bq, wk, bk, wv, bv, wo):
    """Per-core input maps. Slot A of core c = head c; slot B = head 8+c
    (cores 0-3) or a dummy zero head (cores 4-7)."""
    bf16 = ml_dtypes.bfloat16
    # block-major xTb: [128, tt*3072 + j*512 + i] = x[tt*512+i, j*128+p]
    xt = x[0].reshape(NQT, QT, CCH, 128)          # [tt, i, j, p]
    xTb = np.ascontiguousarray(
        xt.transpose(3, 0, 2, 1).reshape(128, NQT * BLK)).astype(bf16)
    masks = np.zeros((4, 128, QT), np.float32)
    dk = np.arange(128)[:, None]
    dq = np.arange(QT)[None, :]
    for p in range(4):
        masks[p] = (dk + 128 * p <= dq)
    masks = masks.astype(bf16)

    in_maps = []
    for c in range(N_CORES):
        hA = c
        hB = 8 + c if c < 4 else None
        w = np.zeros((3, D_MODEL, 128), np.float32)
        b = np.zeros((128, 3), np.float32)
        bvrow = np.zeros((1, 128), np.float32)
        wo2 = np.zeros((128, D_MODEL), np.float32)
        for s, (W, B) in enumerate(((wq, bq), (wk, bk), (wv, bv))):
            w[s, :, 0:64] = W[hA]
            if s < 2:
                b[0:64, s] = B[hA]
            if hB is not None:
                w[s, :, 64:128] = W[hB]
                if s < 2:
                    b[64:128, s] = B[hB]
        bvrow[0, 0:64] = bv[hA]
        if hB is not None:
            bvrow[0, 64:128] = bv[hB]
        wo2[0:64, :] = wo[hA * 64:(hA + 1) * 64, :]
        if hB is not None:
            wo2[64:128, :] = wo[hB * 64:(hB + 1) * 64, :]
        in_maps.append({
            "xTb": xTb,
            "wqkv": w.astype(bf16),
            "bqkv": b.astype(np.float32),
            "bvrow": bvrow.astype(bf16),
            "wo2": wo2.astype(bf16),
            "masks": masks,
        })
    return in_maps


def kernel(_trace=False, _tmpdir=None, **inputs):
    x = np.asarray(inputs["x"], np.float32)
    args = (x,
            np.asarray(inputs["wq"], np.float32), np.asarray(inputs["bq"], np.float32),
            np.asarray(inputs["wk"], np.float32), np.asarray(inputs["bk"], np.float32),
            np.asarray(inputs["wv"], np.float32), np.asarray(inputs["bv"], np.float32),
            np.asarray(inputs["wo"], np.float32))
    bo = np.asarray(inputs["bo"], np.float32)

    if "nc" not in _PROGRAM_CACHE:
        _PROGRAM_CACHE["nc"] = build_program()
    nc = _PROGRAM_CACHE["nc"]

    in_maps = _host_inputs(*args)
    res = run_bass_kernel_spmd(
        nc, in_maps, list(range(N_CORES)), trace=_trace, tmpdir=_tmpdir,
    )
    acc = np.zeros((D_MODEL, T), np.float32)
    for c in range(N_CORES):
        acc += res.results[c]["outT"].astype(np.float32)
    out = acc.T + bo[None, :]
    if _trace:
        return out[None].astype(np.float32), res
    return out[None].astype(np.float32)


# revision 8
# speedup vs baseline: 1.4308x; 1.0675x over previous
"""Multi-head causal attention (B=1, T=4096, D=768, H=12) on 8 trn2 cores.

Sharding: 16 uniform head-slots (2 per core), 12 real heads + 4 dummy
(zero-weight) slots.  Every core runs the IDENTICAL program (SPMD); cores
differ only in the weight data they receive.  Each core computes, for its
two head-slots, the full causal attention over all 4096 tokens plus that
slot-pair's partial output projection (out.T = wo_slice.T @ headout).  The
host sums the 8 partial [768, 4096] fp16 outputs, transposes, adds bias.

Pipeline structure (v4): x arrives in 8 token blocks (block-major DRAM
layout, one 128-line DMA per block; all constants are host-packed so each
needs a single DMA).  Per block tt: project Q/K for those 512 tokens,
build V^T directly via transposed matmuls (lhsT = x chunk), then run
attention for query tile tt.  Softmax normalization is deferred and staged
across later emission points (denominator bounce -> batched reciprocal ->
partition broadcast -> scale + out-projection); the last two query tiles
get their own single-tile rounds, partially emitted between qtile-7
attention groups, to shrink the serial tail.
"""

import math
import numpy as np
import ml_dtypes
from contextlib import ExitStack

import concourse.bass as bass
import concourse.bacc as bacc
import concourse.mybir as mybir
import concourse.tile as tile
from concourse.bass_utils import run_bass_kernel_spmd

BF16 = mybir.dt.bfloat16
F16 = mybir.dt.float16
F32 = mybir.dt.float32
AF = mybir.ActivationFunctionType

T = 4096
D_MODEL = 768
HEAD_DIM = 64
N_HEADS = 12
N_CORES = 8
QT = 512                  # query tile width (one full PSUM bank per chunk)
KC = 128                  # key chunk (psum partition dim)
GRP = 3                   # score chunk-jobs per exp group -> ACT free dim 1536
NQT = T // QT             # 8 query tiles / token blocks
CCH = D_MODEL // 128      # 6 contraction chunks
BLK = QT * CCH            # 3072 cols per block in xTb layout
VST = 208                 # V2 column stride per key chunk

_PROGRAM_CACHE = {}


def build_program():
    nc = bacc.Bacc(None)

    # xTb block-major: xTb[p, tt*3072 + j*512 + i] = x[tt*512 + i, j*128 + p]
    xT_d = nc.declare_dram_parameter("xTb", [128, NQT * BLK], BF16, isOutput=False)
    # w2 pre-packed in SBUF layout: w2[p, (s*6+j)*128 + d] = W_s[j*128+p, d]
    w_d = nc.declare_dram_parameter("w2", [128, 3 * CCH * 128], BF16, isOutput=False)
    b_d = nc.declare_dram_parameter("bqkv", [128, 3], F32, isOutput=False)
    bvr_d = nc.declare_dram_parameter("bvrow", [1, 128], BF16, isOutput=False)
    wo_d = nc.declare_dram_parameter("wo2", [128, D_MODEL], BF16, isOutput=False)
    mk_d = nc.declare_dram_parameter("masks2", [128, 4 * QT], BF16, isOutput=False)
    outT_d = nc.declare_dram_parameter("outT", [D_MODEL, T], F16, isOutput=True)

    with tile.TileContext(nc) as tc, ExitStack() as ctx:
        consts = ctx.enter_context(tc.tile_pool(name="consts", bufs=1))
        big = ctx.enter_context(tc.tile_pool(name="big", bufs=1))
        ptp = ctx.enter_context(tc.tile_pool(name="ptp", bufs=3))
        rp = ctx.enter_context(tc.tile_pool(name="rp", bufs=3))
        osb = ctx.enter_context(tc.tile_pool(name="osb", bufs=3))
        sp = ctx.enter_context(tc.tile_pool(name="sp", bufs=2, space="PSUM"))
        avp = ctx.enter_context(tc.tile_pool(name="avp", bufs=1, space="PSUM"))
        dramp = ctx.enter_context(tc.tile_pool(name="dramp", bufs=2, space="DRAM"))

        xT_sb = big.tile([128, NQT * BLK], BF16, tag="xTb")
        w_sb = consts.tile([128, 3 * CCH * 128], BF16, tag="w")
        b_sb = consts.tile([128, 3], F32, tag="b")
        bvr_sb = consts.tile([1, 128], BF16, tag="bvr")
        wo_sb = consts.tile([128, D_MODEL], BF16, tag="wo")
        mask_sb = consts.tile([128, 4 * QT], BF16, tag="mask")
        ones_sb = consts.tile([1, 128], BF16, tag="ones1")

        # w + b + first x block first so projections start ASAP
        nc.sync.dma_start(out=w_sb[:], in_=w_d[:, :])
        nc.sync.dma_start(out=b_sb[:], in_=b_d[:, :])
        nc.sync.dma_start(out=xT_sb[:, 0:BLK], in_=xT_d[:, 0:BLK])
        nc.sync.dma_start(out=wo_sb[:], in_=wo_d[:, :])
        nc.sync.dma_start(out=mask_sb[:], in_=mk_d[:, :])
        nc.sync.dma_start(out=bvr_sb[:], in_=bvr_d[:, :])
        nc.vector.memset(ones_sb[:], 1.0)

        QT_sb = big.tile([128, T], BF16, tag="qsb")
        KT_sb = big.tile([128, T], BF16, tag="ksb")
        V2 = big.tile([128, (T // KC) * VST], BF16, tag="V2")
        nc.vector.memset(V2[:], 0.0)
        v3 = V2[:].rearrange("p (t c) -> p t c", c=VST)
        nc.vector.memset(v3[:, :, 64:65], 1.0)
        nc.vector.memset(v3[:, :, 97:98], 1.0)
        ho_u = big.tile([128, T], F32, tag="ho_u")      # unnormalized AV
        hob = big.tile([128, T], BF16, tag="hob")       # normalized, bf16
        dn_st = big.tile([128, 2 * QT], F32, tag="dn")  # denom staging

        rounds = {}  # round key -> state

        def xchunk(tt, j, lo, hi):
            base = tt * BLK + j * QT
            return xT_sb[:, base + lo:base + hi]

        def stage_a(qts):
            # denominator rows (dA row 64 bank0, dB row 32 bank1, already
            # staged in dn_st) -> DRAM rd -> partition-spread dn_sb so one
            # [128, 16] reciprocal covers up to 2 qtiles x 2 heads
            n = len(qts)
            c0 = (qts[0] % 2) * QT
            rd = dramp.tile([2, 2 * QT], F32, tag="rd")
            nc.sync.dma_start(out=rd[0:1, 0:n * QT],
                              in_=dn_st[64:65, c0:c0 + n * QT])
            nc.sync.dma_start(out=rd[1:2, 0:n * QT],
                              in_=dn_st[32:33, c0:c0 + n * QT])
            dn_sb = rp.tile([128, 16], F32, tag="dnsb")
            for r in range(2):  # rd row r at linear offset r*2*QT
                nc.sync.dma_start(
                    out=dn_sb[:, r * 8:r * 8 + 4 * n],
                    in_=bass.AP(tensor=rd.tensor, offset=rd.offset + r * 2 * QT,
                                ap=[[4 * n, 128], [1, 4 * n]]))
            rounds[tuple(qts)] = {"dn_sb": dn_sb}

        def stage_b(qts):
            n = len(qts)
            st = rounds[tuple(qts)]
            dn_r = rp.tile([128, 16], F32, tag="dnr")
            nc.vector.reciprocal(dn_r[:, 0:16], st["dn_sb"][:, 0:16])
            rr = dramp.tile([1, 4 * QT], F32, tag="rr")
            for r in range(2):
                nc.sync.dma_start(
                    out=bass.AP(tensor=rr.tensor, offset=rr.offset + r * n * QT,
                                ap=[[4 * n, 128], [1, 4 * n]]),
                    in_=dn_r[:, r * 8:r * 8 + 4 * n])
            # rr linear: [rA(qts[0]) .. rA(qts[n-1]) | rB(qts[0]) .. ]
            rbcs = []
            for i in range(n):
                rbc = rp.tile([128, QT], F32, tag="rbc")
                rA = rr[0:1, i * QT:(i + 1) * QT]
                rB = rr[0:1, (n + i) * QT:(n + i + 1) * QT]
                nc.gpsimd.dma_start(
                    out=rbc[0:64, :],
                    in_=bass.AP(tensor=rA.tensor, offset=rA.offset,
                                ap=[[0, 64]] + list(rA.ap[1:])))
                nc.gpsimd.dma_start(
                    out=rbc[64:128, :],
                    in_=bass.AP(tensor=rB.tensor, offset=rB.offset,
                                ap=[[0, 64]] + list(rB.ap[1:])))
                rbcs.append(rbc)
            st["rbcs"] = rbcs

        def stage_c(qts):
            st = rounds.pop(tuple(qts))
            for i, q2 in enumerate(qts):
                q2s = q2 * QT
                nc.vector.tensor_mul(
                    hob[:, q2s:q2s + QT], ho_u[:, q2s:q2s + QT],
                    st["rbcs"][i][:])
                for dch in range(CCH):
                    op = sp.tile([128, QT], F32, tag="sc")
                    nc.tensor.matmul(
                        op[:], wo_sb[:, dch * 128:(dch + 1) * 128],
                        hob[:, q2s:q2s + QT], start=True, stop=True,
                    )
                    ot = osb.tile([128, QT], F16, tag="ot")
                    nc.vector.tensor_copy(ot[:], op[:])
                    nc.sync.dma_start(
                        out=outT_d[dch * 128:(dch + 1) * 128, q2s:q2s + QT],
                        in_=ot[:],
                    )

        # ================= main token-block pipeline =================
        for tt in range(NQT):
            if tt > 0:
                nc.sync.dma_start(
                    out=xT_sb[:, tt * BLK:(tt + 1) * BLK],
                    in_=xT_d[:, tt * BLK:(tt + 1) * BLK])

            with nc.named_scope("proj"):
                for s, dst in ((0, QT_sb), (1, KT_sb)):
                    pp = sp.tile([128, QT], F32, tag="sc")
                    for j in range(CCH):
                        nc.tensor.matmul(
                            pp[:],
                            w_sb[:, (s * CCH + j) * 128:(s * CCH + j + 1) * 128],
                            xchunk(tt, j, 0, QT),
                            start=(j == 0), stop=(j == CCH - 1),
                        )
                    nc.vector.tensor_scalar_add(
                        dst[:, tt * QT:(tt + 1) * QT], pp[:], b_sb[:, s:s + 1])

            with nc.named_scope("vt"):
                for q4 in range(4):
                    tt4 = tt * 4 + q4
                    vt = sp.tile([128, 128], F32, tag="sc")
                    for j in range(CCH):
                        base = (2 * CCH + j) * 128
                        nc.tensor.matmul(
                            vt[:], xchunk(tt, j, q4 * KC, (q4 + 1) * KC),
                            w_sb[:, base:base + 128],
                            start=(j == 0), stop=False,
                        )
                    nc.tensor.matmul(  # rank-1 bias: out[tok, :] += bv
                        vt[:], ones_sb[:, 0:128], bvr_sb[:, :],
                        start=False, stop=True,
                    )
                    nc.vector.tensor_copy(
                        V2[:, tt4 * VST:tt4 * VST + 64], vt[:, 0:64])
                    nc.vector.tensor_copy(
                        V2[:, tt4 * VST + 129:tt4 * VST + 193], vt[:, 64:128])

            # ---- attention for query tile qi = tt ----
            qi = tt
            qs = qi * QT
            # av bank 0: head-A group (AV rows 0:64, denom row 64)
            # av bank 1: head-B group (denom row 32, AV rows 64:128)
            av = avp.tile([128, 2 * QT], F32, tag="av")
            nsteps = 4 * (qi + 1)
            jobs = [(kc, h) for kc in range(nsteps) for h in (0, 1)]
            # stage work injected between qtile-7 attention groups
            mid = {}
            if qi == 7:
                mid = {4: lambda: stage_b((4, 5)), 8: lambda: stage_b((6,)),
                       12: lambda: stage_c((4, 5)), 16: lambda: stage_c((6,))}
            for gn, g in enumerate(range(0, len(jobs), GRP)):
                if gn in mid:
                    mid[gn]()
                grp = jobs[g:g + GRP]
                width = len(grp) * QT
                with nc.named_scope("score"):
                    sc = sp.tile([128, GRP * QT], F32, tag="sc")
                    for ji, (kc, h) in enumerate(grp):
                        nc.tensor.matmul(
                            sc[:, ji * QT:(ji + 1) * QT],
                            KT_sb[64 * h:64 * h + 64, kc * KC:(kc + 1) * KC],
                            QT_sb[64 * h:64 * h + 64, qs:qs + QT],
                            start=True, stop=True, tile_position=(64 * h, 0),
                        )
                pt = ptp.tile([128, GRP * QT], BF16, tag="pt")
                with nc.named_scope("exp"):
                    nc.scalar.activation(
                        pt[:, :width], sc[:, :width], AF.Exp,
                        scale=1.0 / math.sqrt(HEAD_DIM),
                    )
                with nc.named_scope("av"):
                    for ji, (kc, h) in enumerate(grp):
                        ptj = pt[:, ji * QT:(ji + 1) * QT]
                        if kc >= 4 * qi:  # diagonal straddle
                            pat = kc - 4 * qi
                            m = mask_sb[:, pat * QT:(pat + 1) * QT]
                            nc.vector.tensor_mul(ptj, ptj, m)
                        st_ = kc == 0
                        sp_ = kc == nsteps - 1
                        vbase = kc * VST
                        if h == 0:
                            # lhsT padded to 128 cols; rows 65:128 junk
                            nc.tensor.matmul(
                                av[0:128, 0:QT], V2[:, vbase:vbase + 128], ptj,
                                start=st_, stop=sp_, tile_position=(0, 0),
                            )
                        else:
                            nc.tensor.matmul(
                                av[0:128, QT:2 * QT],
                                V2[:, vbase + 65:vbase + 193],
                                ptj, start=st_, stop=sp_, tile_position=(0, 0),
                            )
            # stash unnormalized AV (DVE) + denominators (ScalarE, parallel)
            with nc.named_scope("stash"):
                dcol = (qi % 2) * QT
                nc.scalar.copy(dn_st[64:65, dcol:dcol + QT], av[64:65, 0:QT])
                nc.scalar.copy(dn_st[32:33, dcol:dcol + QT],
                               av[32:33, QT:2 * QT])
                nc.vector.tensor_copy(ho_u[0:64, qs:qs + QT], av[0:64, 0:QT])
                nc.vector.tensor_copy(
                    ho_u[64:128, qs:qs + QT], av[64:128, QT:2 * QT])

            # staged deferred normalization + out-projection
            with nc.named_scope("norm"):
                if qi == 1:
                    stage_a((0, 1))
                elif qi == 2:
                    stage_b((0, 1))
                elif qi == 3:
                    stage_c((0, 1))
                    stage_a((2, 3))
                elif qi == 4:
                    stage_b((2, 3))
                elif qi == 5:
                    stage_c((2, 3))
                    stage_a((4, 5))
                elif qi == 6:
                    stage_a((6,))
                elif qi == 7:
                    stage_a((7,))
                    stage_b((7,))
                    stage_c((7,))
    nc.finalize()
    return nc


def _host_inputs(x, wq, bq, wk, bk, wv, bv, wo):
    """Per-core input maps. Slot A of core c = head c; slot B = head 8+c
    (cores 0-3) or a dummy zero head (cores 4-7)."""
    bf16 = ml_dtypes.bfloat16
    # block-major xTb: [128, tt*3072 + j*512 + i] = x[tt*512+i, j*128+p]
    xt = x[0].reshape(NQT, QT, CCH, 128)          # [tt, i, j, p]
    xTb = np.ascontiguousarray(
        xt.transpose(3, 0, 2, 1).reshape(128, NQT * BLK)).astype(bf16)
    masks = np.zeros((4, 128, QT), np.float32)
    dk = np.arange(128)[:, None]
    dq = np.arange(QT)[None, :]
    for p in range(4):
        masks[p] = (dk + 128 * p <= dq)
    masks2 = np.ascontiguousarray(
        masks.transpose(1, 0, 2).reshape(128, 4 * QT)).astype(bf16)

    in_maps = []
    for c in range(N_CORES):
        hA = c
        hB = 8 + c if c < 4 else None
        w = np.zeros((3, D_MODEL, 128), np.float32)
        b = np.zeros((128, 3), np.float32)
        bvrow = np.zeros((1, 128), np.float32)
        wo2 = np.zeros((128, D_MODEL), np.float32)
        for s, (W, B) in enumerate(((wq, bq), (wk, bk), (wv, bv))):
            w[s, :, 0:64] = W[hA]
            if s < 2:
                b[0:64, s] = B[hA]
            if hB is not None:
                w[s, :, 64:128] = W[hB]
                if s < 2:
                    b[64:128, s] = B[hB]
        bvrow[0, 0:64] = bv[hA]
        if hB is not None:
            bvrow[0, 64:128] = bv[hB]
        wo2[0:64, :] = wo[hA * 64:(hA + 1) * 64, :]
        if hB is not None:
            wo2[64:128, :] = wo[hB * 64:(hB + 1) * 64, :]
        # w2[p, (s*6+j)*128 + d] = w[s, j*128+p, d]
        w2 = np.ascontiguousarray(
            w.reshape(3, CCH, 128, 128).transpose(2, 0, 1, 3)
            .reshape(128, 3 * CCH * 128)).astype(bf16)
        in_maps.append({
            "xTb": xTb,
            "w2": w2,
            "bqkv": b.astype(np.float32),
            "bvrow": bvrow.astype(bf16),
            "wo2": wo2.astype(bf16),
            "masks2": masks2,
        })
    return in_maps


def kernel(_trace=False, _tmpdir=None, **inputs):
    x = np.asarray(inputs["x"], np.float32)
    args = (x,
            np.asarray(inputs["wq"], np.float32), np.asarray(inputs["bq"], np.float32),
            np.asarray(inputs["wk"], np.float32), np.asarray(inputs["bk"], np.float32),
            np.asarray(inputs["wv"], np.float32), np.asarray(inputs["bv"], np.float32),
            np.asarray(inputs["wo"], np.float32))
    bo = np.asarray(inputs["bo"], np.float32)

    if "nc" not in _PROGRAM_CACHE:
        _PROGRAM_CACHE["nc"] = build_program()
    nc = _PROGRAM_CACHE["nc"]

    in_maps = _host_inputs(*args)
    res = run_bass_kernel_spmd(
        nc, in_maps, list(range(N_CORES)), trace=_trace, tmpdir=_tmpdir,
    )
    acc = np.zeros((D_MODEL, T), np.float32)
    for c in range(N_CORES):
        acc += res.results[c]["outT"].astype(np.float32)
    out = acc.T + bo[None, :]
    if _trace:
        return out[None].astype(np.float32), res
    return out[None].astype(np.float32)


# revision 13
# speedup vs baseline: 1.4457x; 1.0104x over previous
"""Multi-head causal attention (B=1, T=4096, D=768, H=12) on 8 trn2 cores.

Sharding: 16 uniform head-slots (2 per core), 12 real heads + 4 dummy
(zero-weight) slots.  Every core runs the IDENTICAL program (SPMD); cores
differ only in the weight data they receive.  Each core computes, for its
two head-slots, the full causal attention over all 4096 tokens plus that
slot-pair's partial output projection (out.T = wo_slice.T @ headout).  The
host sums the 8 partial [768, 4096] fp16 outputs, transposes, adds bias.

Pipeline structure (v4): x arrives in 8 token blocks (block-major DRAM
layout, one 128-line DMA per block; all constants are host-packed so each
needs a single DMA).  Per block tt: project Q/K for those 512 tokens,
build V^T directly via transposed matmuls (lhsT = x chunk), then run
attention for query tile tt.  Softmax normalization is deferred and staged
across later emission points (denominator bounce -> batched reciprocal ->
partition broadcast -> scale + out-projection); the last two query tiles
get their own single-tile rounds, partially emitted between qtile-7
attention groups, to shrink the serial tail.
"""

import math
import numpy as np
import ml_dtypes
from contextlib import ExitStack

import concourse.bass as bass
import concourse.bacc as bacc
import concourse.mybir as mybir
import concourse.tile as tile
from concourse.bass_utils import run_bass_kernel_spmd

BF16 = mybir.dt.bfloat16
F16 = mybir.dt.float16
F32 = mybir.dt.float32
AF = mybir.ActivationFunctionType

T = 4096
D_MODEL = 768
HEAD_DIM = 64
N_HEADS = 12
N_CORES = 8
QT = 512                  # query tile width (one full PSUM bank per chunk)
KC = 128                  # key chunk (psum partition dim)
GRP = 3                   # score chunk-jobs per exp group -> ACT free dim 1536
NQT = T // QT             # 8 query tiles / token blocks
CCH = D_MODEL // 128      # 6 contraction chunks
BLK = QT * CCH            # 3072 cols per block in xTb layout
VST = 208                 # V2 column stride per key chunk

_PROGRAM_CACHE = {}


def build_program():
    nc = bacc.Bacc(None)

    # xTb block-major: xTb[p, tt*3072 + j*512 + i] = x[tt*512 + i, j*128 + p]
    xT_d = nc.declare_dram_parameter("xTb", [128, NQT * BLK], BF16, isOutput=False)
    # w2 pre-packed in SBUF layout: w2[p, (s*6+j)*128 + d] = W_s[j*128+p, d]
    w_d = nc.declare_dram_parameter("w2", [128, 3 * CCH * 128], BF16, isOutput=False)
    b_d = nc.declare_dram_parameter("bqkv", [128, 3], F32, isOutput=False)
    bvr_d = nc.declare_dram_parameter("bvrow", [1, 128], BF16, isOutput=False)
    wo_d = nc.declare_dram_parameter("wo2", [128, D_MODEL], BF16, isOutput=False)
    mk_d = nc.declare_dram_parameter("masks2", [128, 4 * QT], BF16, isOutput=False)
    outT_d = nc.declare_dram_parameter("outT", [D_MODEL, T], F16, isOutput=True)

    with tile.TileContext(nc) as tc, ExitStack() as ctx:
        consts = ctx.enter_context(tc.tile_pool(name="consts", bufs=1))
        big = ctx.enter_context(tc.tile_pool(name="big", bufs=1))
        ptp = ctx.enter_context(tc.tile_pool(name="ptp", bufs=4))
        rp = ctx.enter_context(tc.tile_pool(name="rp", bufs=3))
        osb = ctx.enter_context(tc.tile_pool(name="osb", bufs=4))
        sp = ctx.enter_context(tc.tile_pool(name="sp", bufs=2, space="PSUM"))
        avp = ctx.enter_context(tc.tile_pool(name="avp", bufs=1, space="PSUM"))
        dramp = ctx.enter_context(tc.tile_pool(name="dramp", bufs=2, space="DRAM"))

        xT_sb = big.tile([128, NQT * BLK], BF16, tag="xTb")
        w_sb = consts.tile([128, 3 * CCH * 128], BF16, tag="w")
        b_sb = consts.tile([128, 3], F32, tag="b")
        bvr_sb = consts.tile([1, 128], BF16, tag="bvr")
        wo_sb = consts.tile([128, D_MODEL], BF16, tag="wo")
        mask_sb = consts.tile([128, 4 * QT], BF16, tag="mask")
        ones_sb = consts.tile([1, QT], BF16, tag="ones1")

        # w + b + first x block first so projections start ASAP
        nc.sync.dma_start(out=w_sb[:], in_=w_d[:, :])
        nc.sync.dma_start(out=b_sb[:], in_=b_d[:, :])
        nc.sync.dma_start(out=xT_sb[:, 0:BLK], in_=xT_d[:, 0:BLK])
        nc.sync.dma_start(out=wo_sb[:], in_=wo_d[:, :])
        nc.sync.dma_start(out=mask_sb[:], in_=mk_d[:, :])
        nc.sync.dma_start(out=bvr_sb[:], in_=bvr_d[:, :])
        nc.vector.memset(ones_sb[:], 1.0)

        # warm the PE (HAM clock gate) during the input-DMA window with
        # throwaway rank-1 matmuls; results land in a psum tile nobody reads
        warm = avp.tile([128, 2 * QT], F32, tag="av")
        for _ in range(28):
            nc.tensor.matmul(
                warm[0:128, 0:QT], ones_sb[:, 0:128], ones_sb[:, 0:QT],
                start=True, stop=True,
            )

        QT_sb = big.tile([128, T], BF16, tag="qsb")
        KT_sb = big.tile([128, T], BF16, tag="ksb")
        V2 = big.tile([128, (T // KC) * VST], BF16, tag="V2")
        nc.vector.memset(V2[:], 0.0)
        v3 = V2[:].rearrange("p (t c) -> p t c", c=VST)
        nc.vector.memset(v3[:, :, 64:65], 1.0)
        nc.vector.memset(v3[:, :, 97:98], 1.0)
        ho_u = big.tile([128, T], F32, tag="ho_u")      # unnormalized AV
        hob = big.tile([128, T], BF16, tag="hob")       # normalized, bf16
        dn_st = big.tile([128, 2 * QT], F32, tag="dn")  # denom staging

        rounds = {}  # round key -> state

        def xchunk(tt, j, lo, hi):
            base = tt * BLK + j * QT
            return xT_sb[:, base + lo:base + hi]

        def stage_a(qts):
            # denominator rows (dA row 64 bank0, dB row 32 bank1, already
            # staged in dn_st) -> DRAM rd -> partition-spread dn_sb so one
            # [128, 16] reciprocal covers up to 2 qtiles x 2 heads
            n = len(qts)
            c0 = (qts[0] % 2) * QT
            rd = dramp.tile([2, 2 * QT], F32, tag="rd")
            nc.sync.dma_start(out=rd[0:1, 0:n * QT],
                              in_=dn_st[64:65, c0:c0 + n * QT])
            nc.sync.dma_start(out=rd[1:2, 0:n * QT],
                              in_=dn_st[32:33, c0:c0 + n * QT])
            dn_sb = rp.tile([128, 16], F32, tag="dnsb")
            for r in range(2):  # rd row r at linear offset r*2*QT
                nc.sync.dma_start(
                    out=dn_sb[:, r * 8:r * 8 + 4 * n],
                    in_=bass.AP(tensor=rd.tensor, offset=rd.offset + r * 2 * QT,
                                ap=[[4 * n, 128], [1, 4 * n]]))
            rounds[tuple(qts)] = {"dn_sb": dn_sb}

        def stage_b(qts):
            n = len(qts)
            st = rounds[tuple(qts)]
            dn_r = rp.tile([128, 16], F32, tag="dnr")
            nc.vector.reciprocal(dn_r[:, 0:16], st["dn_sb"][:, 0:16])
            rr = dramp.tile([1, 4 * QT], F32, tag="rr")
            for r in range(2):
                nc.sync.dma_start(
                    out=bass.AP(tensor=rr.tensor, offset=rr.offset + r * n * QT,
                                ap=[[4 * n, 128], [1, 4 * n]]),
                    in_=dn_r[:, r * 8:r * 8 + 4 * n])
            # rr linear: [rA(qts[0]) .. rA(qts[n-1]) | rB(qts[0]) .. ]
            rbcs = []
            for i in range(n):
                rbc = rp.tile([128, QT], F32, tag="rbc")
                rA = rr[0:1, i * QT:(i + 1) * QT]
                rB = rr[0:1, (n + i) * QT:(n + i + 1) * QT]
                nc.gpsimd.dma_start(
                    out=rbc[0:64, :],
                    in_=bass.AP(tensor=rA.tensor, offset=rA.offset,
                                ap=[[0, 64]] + list(rA.ap[1:])))
                nc.gpsimd.dma_start(
                    out=rbc[64:128, :],
                    in_=bass.AP(tensor=rB.tensor, offset=rB.offset,
                                ap=[[0, 64]] + list(rB.ap[1:])))
                rbcs.append(rbc)
            st["rbcs"] = rbcs

        def stage_c(qts):
            st = rounds.pop(tuple(qts))
            for i, q2 in enumerate(qts):
                q2s = q2 * QT
                nc.vector.tensor_mul(
                    hob[:, q2s:q2s + QT], ho_u[:, q2s:q2s + QT],
                    st["rbcs"][i][:])
                for dch in range(CCH):
                    op = sp.tile([128, QT], F32, tag="sc")
                    nc.tensor.matmul(
                        op[:], wo_sb[:, dch * 128:(dch + 1) * 128],
                        hob[:, q2s:q2s + QT], start=True, stop=True,
                    )
                    ot = osb.tile([128, QT], F16, tag="ot")
                    nc.vector.tensor_copy(ot[:], op[:])
                    nc.sync.dma_start(
                        out=outT_d[dch * 128:(dch + 1) * 128, q2s:q2s + QT],
                        in_=ot[:],
                    )

        # ================= main token-block pipeline =================
        for tt in range(NQT):
            if tt > 0:
                nc.sync.dma_start(
                    out=xT_sb[:, tt * BLK:(tt + 1) * BLK],
                    in_=xT_d[:, tt * BLK:(tt + 1) * BLK])

            with nc.named_scope("proj"):
                for s, dst in ((0, QT_sb), (1, KT_sb)):
                    pp = sp.tile([128, QT], F32, tag="sc")
                    for j in range(CCH):
                        nc.tensor.matmul(
                            pp[:],
                            w_sb[:, (s * CCH + j) * 128:(s * CCH + j + 1) * 128],
                            xchunk(tt, j, 0, QT),
                            start=(j == 0), stop=(j == CCH - 1),
                        )
                    nc.vector.tensor_scalar_add(
                        dst[:, tt * QT:(tt + 1) * QT], pp[:], b_sb[:, s:s + 1])

            with nc.named_scope("vt"):
                for q4 in range(4):
                    tt4 = tt * 4 + q4
                    vt = sp.tile([128, 128], F32, tag="sc")
                    for j in range(CCH):
                        base = (2 * CCH + j) * 128
                        nc.tensor.matmul(
                            vt[:], xchunk(tt, j, q4 * KC, (q4 + 1) * KC),
                            w_sb[:, base:base + 128],
                            start=(j == 0), stop=False,
                        )
                    nc.tensor.matmul(  # rank-1 bias: out[tok, :] += bv
                        vt[:], ones_sb[:, 0:128], bvr_sb[:, :],
                        start=False, stop=True,
                    )
                    nc.vector.tensor_copy(
                        V2[:, tt4 * VST:tt4 * VST + 64], vt[:, 0:64])
                    nc.vector.tensor_copy(
                        V2[:, tt4 * VST + 129:tt4 * VST + 193], vt[:, 64:128])

            # ---- attention for query tile qi = tt ----
            qi = tt
            qs = qi * QT
            # av bank 0: head-A group (AV rows 0:64, denom row 64)
            # av bank 1: head-B group (denom row 32, AV rows 64:128)
            av = avp.tile([128, 2 * QT], F32, tag="av")
            nsteps = 4 * (qi + 1)
            jobs = [(kc, h) for kc in range(nsteps) for h in (0, 1)]
            # stage work injected between qtile-7 attention groups
            mid = {}
            if qi == 7:
                mid = {4: lambda: stage_b((4, 5)), 8: lambda: stage_b((6,)),
                       12: lambda: stage_c((4, 5)), 16: lambda: stage_c((6,))}
            for gn, g in enumerate(range(0, len(jobs), GRP)):
                if gn in mid:
                    mid[gn]()
                grp = jobs[g:g + GRP]
                width = len(grp) * QT
                with nc.named_scope("score"):
                    sc = sp.tile([128, GRP * QT], F32, tag="sc")
                    for ji, (kc, h) in enumerate(grp):
                        nc.tensor.matmul(
                            sc[:, ji * QT:(ji + 1) * QT],
                            KT_sb[64 * h:64 * h + 64, kc * KC:(kc + 1) * KC],
                            QT_sb[64 * h:64 * h + 64, qs:qs + QT],
                            start=True, stop=True, tile_position=(64 * h, 0),
                        )
                pt = ptp.tile([128, GRP * QT], BF16, tag="pt")
                with nc.named_scope("exp"):
                    nc.scalar.activation(
                        pt[:, :width], sc[:, :width], AF.Exp,
                        scale=1.0 / math.sqrt(HEAD_DIM),
                    )
                with nc.named_scope("av"):
                    for ji, (kc, h) in enumerate(grp):
                        if kc >= 4 * qi:  # diagonal straddle
                            ptj = pt[:, ji * QT:(ji + 1) * QT]
                            pat = kc - 4 * qi
                            m = mask_sb[:, pat * QT:(pat + 1) * QT]
                            nc.vector.tensor_mul(ptj, ptj, m)
                    for ji, (kc, h) in enumerate(grp):
                        ptj = pt[:, ji * QT:(ji + 1) * QT]
                        st_ = kc == 0
                        sp_ = kc == nsteps - 1
                        vbase = kc * VST
                        if h == 0:
                            # lhsT padded to 128 cols; rows 65:128 junk
                            nc.tensor.matmul(
                                av[0:128, 0:QT], V2[:, vbase:vbase + 128], ptj,
                                start=st_, stop=sp_, tile_position=(0, 0),
                            )
                        else:
                            nc.tensor.matmul(
                                av[0:128, QT:2 * QT],
                                V2[:, vbase + 65:vbase + 193],
                                ptj, start=st_, stop=sp_, tile_position=(0, 0),
                            )
            # stash unnormalized AV (DVE) + denominators (ScalarE, parallel)
            with nc.named_scope("stash"):
                dcol = (qi % 2) * QT
                nc.scalar.copy(dn_st[64:65, dcol:dcol + QT], av[64:65, 0:QT])
                nc.scalar.copy(dn_st[32:33, dcol:dcol + QT],
                               av[32:33, QT:2 * QT])
                nc.vector.tensor_copy(ho_u[0:64, qs:qs + QT], av[0:64, 0:QT])
                nc.vector.tensor_copy(
                    ho_u[64:128, qs:qs + QT], av[64:128, QT:2 * QT])

            # staged deferred normalization + out-projection
            with nc.named_scope("norm"):
                if qi == 1:
                    stage_a((0, 1))
                elif qi == 2:
                    stage_b((0, 1))
                elif qi == 3:
                    stage_c((0, 1))
                    stage_a((2, 3))
                elif qi == 4:
                    stage_b((2, 3))
                elif qi == 5:
                    stage_c((2, 3))
                    stage_a((4, 5))
                elif qi == 6:
                    stage_a((6,))
                elif qi == 7:
                    stage_a((7,))
                    stage_b((7,))
                    stage_c((7,))
    nc.finalize()
    return nc


def _host_inputs(x, wq, bq, wk, bk, wv, bv, wo):
    """Per-core input maps. Slot A of core c = head c; slot B = head 8+c
    (cores 0-3) or a dummy zero head (cores 4-7)."""
    bf16 = ml_dtypes.bfloat16
    # block-major xTb: [128, tt*3072 + j*512 + i] = x[tt*512+i, j*128+p]
    xt = x[0].reshape(NQT, QT, CCH, 128)          # [tt, i, j, p]
    xTb = np.ascontiguousarray(
        xt.transpose(3, 0, 2, 1).reshape(128, NQT * BLK)).astype(bf16)
    masks = np.zeros((4, 128, QT), np.float32)
    dk = np.arange(128)[:, None]
    dq = np.arange(QT)[None, :]
    for p in range(4):
        masks[p] = (dk + 128 * p <= dq)
    masks2 = np.ascontiguousarray(
        masks.transpose(1, 0, 2).reshape(128, 4 * QT)).astype(bf16)

    in_maps = []
    for c in range(N_CORES):
        hA = c
        hB = 8 + c if c < 4 else None
        w = np.zeros((3, D_MODEL, 128), np.float32)
        b = np.zeros((128, 3), np.float32)
        bvrow = np.zeros((1, 128), np.float32)
        wo2 = np.zeros((128, D_MODEL), np.float32)
        for s, (W, B) in enumerate(((wq, bq), (wk, bk), (wv, bv))):
            w[s, :, 0:64] = W[hA]
            if s < 2:
                b[0:64, s] = B[hA]
            if hB is not None:
                w[s, :, 64:128] = W[hB]
                if s < 2:
                    b[64:128, s] = B[hB]
        bvrow[0, 0:64] = bv[hA]
        if hB is not None:
            bvrow[0, 64:128] = bv[hB]
        wo2[0:64, :] = wo[hA * 64:(hA + 1) * 64, :]
        if hB is not None:
            wo2[64:128, :] = wo[hB * 64:(hB + 1) * 64, :]
        # w2[p, (s*6+j)*128 + d] = w[s, j*128+p, d]
        w2 = np.ascontiguousarray(
            w.reshape(3, CCH, 128, 128).transpose(2, 0, 1, 3)
            .reshape(128, 3 * CCH * 128)).astype(bf16)
        in_maps.append({
            "xTb": xTb,
            "w2": w2,
            "bqkv": b.astype(np.float32),
            "bvrow": bvrow.astype(bf16),
            "wo2": wo2.astype(bf16),
            "masks2": masks2,
        })
    return in_maps


def kernel(_trace=False, _tmpdir=None, **inputs):
    x = np.asarray(inputs["x"], np.float32)
    args = (x,
            np.asarray(inputs["wq"], np.float32), np.asarray(inputs["bq"], np.float32),
            np.asarray(inputs["wk"], np.float32), np.asarray(inputs["bk"], np.float32),
            np.asarray(inputs["wv"], np.float32), np.asarray(inputs["bv"], np.float32),
            np.asarray(inputs["wo"], np.float32))
    bo = np.asarray(inputs["bo"], np.float32)

    if "nc" not in _PROGRAM_CACHE:
        _PROGRAM_CACHE["nc"] = build_program()
    nc = _PROGRAM_CACHE["nc"]

    in_maps = _host_inputs(*args)
    res = run_bass_kernel_spmd(
        nc, in_maps, list(range(N_CORES)), trace=_trace, tmpdir=_tmpdir,
    )
    acc = np.zeros((D_MODEL, T), np.float32)
    for c in range(N_CORES):
        acc += res.results[c]["outT"].astype(np.float32)
    out = acc.T + bo[None, :]
    if _trace:
        return out[None].astype(np.float32), res
    return out[None].astype(np.float32)
